# revision 3
# baseline (speedup 1.0000x reference)
"""Trainium2 Bass kernel for the AlpacaMH head.

Math (per sample b, per (z,u) pair, A = Linv[b,z,u], 128x128):
    phi = MLP_encoder(x[b])                       # (P,)
    s_zu = A^T phi                                # one PE matvec, A stationary
    sigma_raw[zu] = phi . s_zu     (= phi^T A phi)
    mu[zu]        = q_zu . s_zu    (= phi^T A q)
    cov[zu] = exp(logSigEps[u]) * (1 + sigma_raw[zu])

Strategy: pure data-parallel over batch across 8 NeuronCores (8 samples
per core).  The 256 MiB Linv tensor is streamed from HBM once, converted
f32->bf16 (spread across DVE/ACT/Pool engines), and contracted on the
TensorEngine with A as the bf16 stationary operand (fast weight load)
and phi as the N=1 moving operand; the [128,1] outputs land as PSUM
columns, building S_phi[128, 64] per sample.  A second tiny matmul with
lhsT=phi reduces S_phi to sigma, and a DVE multiply + ones-matmul
reduces Q^T * S_phi to mu.  DMA of Linv is the roofline (~33.5 MB/core).
"""

import numpy as np

import concourse.bass as bass  # noqa: F401  (registers engine classes)
import concourse.mybir as mybir
import concourse.tile as tile
from concourse import bacc
from concourse.masks import make_identity

F32 = mybir.dt.float32
BF16 = mybir.dt.bfloat16
AF = mybir.ActivationFunctionType
ALU = mybir.AluOpType

# Problem dims (hardcoded per spec)
B, Z, U, P, X, H = 64, 8, 8, 128, 64, 512
NCORES = 8
BS = B // NCORES          # samples per core
ZU = Z * U                # 64 (z,u) pairs per sample
CHUNK = 16                # zu pairs per DMA/convert chunk
NCHUNK = ZU // CHUNK


def build_nc():
    nc = bacc.Bacc(None, target_bir_lowering=False, debug=False)
    with tile.TileContext(nc) as tc:
        with (
            tc.tile_pool(name="dram", bufs=1, space="DRAM") as dram,
            tc.tile_pool(name="const", bufs=1) as const,
            tc.tile_pool(name="wts", bufs=1) as wts,
            tc.tile_pool(name="lf32", bufs=4) as lf32p,
            tc.tile_pool(name="lbf", bufs=4) as lbfp,
            tc.tile_pool(name="small", bufs=2) as small,
            tc.tile_pool(name="epsum", bufs=2, space="PSUM") as epsum,
            tc.tile_pool(name="wpsum", bufs=3, space="PSUM") as wpsum,
            tc.tile_pool(name="qpsum", bufs=1, space="PSUM") as qpsum,
            tc.tile_pool(name="rpsum", bufs=2, space="PSUM") as rpsum,
        ):
            # ---- DRAM parameters (names must match in_maps keys) ----
            x_d = dram.tile([BS, X], F32, kind="ExternalInput", name="x", uniquify=False)
            linv_d = dram.tile([BS, Z, U, P, P], F32, kind="ExternalInput", name="Linv", uniquify=False)
            q_d = dram.tile([BS, Z, U, 1, P], F32, kind="ExternalInput", name="Q", uniquify=False)
            w1_d = dram.tile([X, H], F32, kind="ExternalInput", name="W1", uniquify=False)
            b1_d = dram.tile([H], F32, kind="ExternalInput", name="b1", uniquify=False)
            w2_d = dram.tile([H, H], F32, kind="ExternalInput", name="W2", uniquify=False)
            b2_d = dram.tile([H], F32, kind="ExternalInput", name="b2", uniquify=False)
            w3_d = dram.tile([H, H], F32, kind="ExternalInput", name="W3", uniquify=False)
            b3_d = dram.tile([H], F32, kind="ExternalInput", name="b3", uniquify=False)
            w4_d = dram.tile([H, P], F32, kind="ExternalInput", name="W4", uniquify=False)
            b4_d = dram.tile([P], F32, kind="ExternalInput", name="b4", uniquify=False)
            ls_d = dram.tile([U], F32, kind="ExternalInput", name="logSigEps", uniquify=False)
            mu_d = dram.tile([BS, Z, U, 1], F32, kind="ExternalOutput", name="mu_out", uniquify=False)
            cov_d = dram.tile([BS, Z, U], F32, kind="ExternalOutput", name="cov_out", uniquify=False)

            # ---- constants / weights to SBUF ----
            ident = const.tile([128, 128], F32)
            make_identity(nc, ident[:])
            ones_bf = const.tile([128, 1], BF16)
            nc.vector.memset(ones_bf[:], 1.0)

            w1_sb = wts.tile([X, H], F32)
            nc.sync.dma_start(out=w1_sb[:], in_=w1_d[:])
            w2_sb = wts.tile([128, 4, H], F32)
            nc.sync.dma_start(out=w2_sb[:], in_=w2_d[:].rearrange("(kk p) h -> p kk h", p=128))
            w3_sb = wts.tile([128, 4, H], F32)
            nc.sync.dma_start(out=w3_sb[:], in_=w3_d[:].rearrange("(kk p) h -> p kk h", p=128))
            w4_sb = wts.tile([128, 4, P], F32)
            nc.sync.dma_start(out=w4_sb[:], in_=w4_d[:].rearrange("(kk p) h -> p kk h", p=128))
            b1_sb = wts.tile([128, 4], F32)
            nc.sync.dma_start(out=b1_sb[:], in_=b1_d[:].rearrange("(c p) -> p c", p=128))
            b2_sb = wts.tile([128, 4], F32)
            nc.sync.dma_start(out=b2_sb[:], in_=b2_d[:].rearrange("(c p) -> p c", p=128))
            b3_sb = wts.tile([128, 4], F32)
            nc.sync.dma_start(out=b3_sb[:], in_=b3_d[:].rearrange("(c p) -> p c", p=128))
            b4_sb = wts.tile([P, 1], F32)
            nc.sync.dma_start(out=b4_sb[:], in_=b4_d[:, None])

            # sigma eps factor as a row, tiled to (s, z, u) on partition 0
            ses = wts.tile([1, U], F32)
            nc.sync.dma_start(out=ses[:], in_=ls_d[None, :])
            sigf_row = const.tile([1, U], F32)
            nc.scalar.activation(sigf_row[:], ses[:], AF.Exp)

            # ---- encoder: phi = linear(elu(elu(elu(x@W1+b1)@W2+b2)@W3+b3)@W4+b4)
            # activations kept transposed: [feature partitions, batch free]
            x_sb = small.tile([BS, X], F32, tag="x_sb")
            nc.sync.dma_start(out=x_sb[:], in_=x_d[:])
            xt_ps = epsum.tile([X, BS], F32, tag="enc")
            nc.tensor.transpose(xt_ps[:], x_sb[:], ident[:BS, :BS])
            xt = small.tile([X, BS], F32, tag="xt")
            nc.scalar.copy(xt[:], xt_ps[:])

            def elu_into(pre_ps, bias_col, out_ap):
                # out = elu(pre + bias) = max(v, exp(min(v,0)) - 1)
                v = small.tile([128, BS], F32, tag="elu_v")
                nc.vector.tensor_scalar_add(v[:], pre_ps[:], bias_col)
                m = small.tile([128, BS], F32, tag="elu_m")
                nc.vector.tensor_scalar_min(m[:], v[:], 0.0)
                e = small.tile([128, BS], F32, tag="elu_e")
                nc.scalar.activation(e[:], m[:], AF.Exp)
                nc.vector.tensor_scalar_add(e[:], e[:], -1.0)
                nc.vector.tensor_tensor(out_ap, v[:], e[:], op=ALU.max)

            h1 = small.tile([128, 4, BS], F32, tag="h1")
            for m in range(4):
                ps = epsum.tile([128, BS], F32, tag="enc")
                nc.tensor.matmul(ps[:], w1_sb[:, m * 128:(m + 1) * 128], xt[:],
                                 start=True, stop=True)
                elu_into(ps, b1_sb[:, m:m + 1], h1[:, m, :])
            h2 = small.tile([128, 4, BS], F32, tag="h2")
            for m in range(4):
                ps = epsum.tile([128, BS], F32, tag="enc")
                for kk in range(4):
                    nc.tensor.matmul(ps[:], w2_sb[:, kk, m * 128:(m + 1) * 128],
                                     h1[:, kk, :], start=(kk == 0), stop=(kk == 3))
                elu_into(ps, b2_sb[:, m:m + 1], h2[:, m, :])
            h3 = small.tile([128, 4, BS], F32, tag="h3")
            for m in range(4):
                ps = epsum.tile([128, BS], F32, tag="enc")
                for kk in range(4):
                    nc.tensor.matmul(ps[:], w3_sb[:, kk, m * 128:(m + 1) * 128],
                                     h2[:, kk, :], start=(kk == 0), stop=(kk == 3))
                elu_into(ps, b3_sb[:, m:m + 1], h3[:, m, :])

            phi_ps = epsum.tile([P, BS], F32, tag="enc")
            for kk in range(4):
                nc.tensor.matmul(phi_ps[:], w4_sb[:, kk, :], h3[:, kk, :],
                                 start=(kk == 0), stop=(kk == 3))
            phiT = small.tile([P, BS], F32, tag="phiT")
            nc.vector.tensor_scalar_add(phiT[:], phi_ps[:], b4_sb[:])
            phiT_bf = const.tile([P, BS], BF16)
            nc.vector.tensor_copy(phiT_bf[:], phiT[:])

            # ---- main stream ----
            lv = linv_d[:].rearrange("s z u i j -> s i (z u) j")      # [BS, 128, ZU, P]
            qv = q_d[:].rearrange("s z u one j -> s (z u) (one j)")   # [BS, ZU, P]
            mu_collect = const.tile([1, BS * ZU], F32)
            sig_collect = const.tile([1, BS * ZU], F32)

            ci = 0
            for s in range(BS):
                # Q[s]^T: [zu, j] -> [j, zu] via PE transpose
                qs = small.tile([ZU, P], F32, tag="qs")
                nc.sync.dma_start(out=qs[:], in_=qv[s])
                qt_ps = qpsum.tile([P, ZU], F32, tag="qt")
                nc.tensor.transpose(qt_ps[:], qs[:], ident[:ZU, :ZU])
                qt = small.tile([P, ZU], F32, tag="qt_sb")
                nc.scalar.copy(qt[:], qt_ps[:])

                sphi = wpsum.tile([P, ZU], F32, tag="W")
                for k in range(NCHUNK):
                    lf = lf32p.tile([128, CHUNK, P], F32, tag="lf")
                    nc.sync.dma_start(out=lf[:], in_=lv[s, :, k * CHUNK:(k + 1) * CHUNK, :])
                    lb = lbfp.tile([128, CHUNK, P], BF16, tag="lb")
                    eng = ci % 3
                    ci += 1
                    if eng == 0:
                        nc.vector.tensor_copy(lb[:], lf[:])
                    elif eng == 1:
                        nc.scalar.copy(lb[:], lf[:])
                    else:
                        nc.gpsimd.tensor_copy(lb[:], lf[:])
                    for t in range(CHUNK):
                        zu = k * CHUNK + t
                        nc.tensor.matmul(sphi[:, zu:zu + 1], lb[:, t, :],
                                         phiT_bf[:, s:s + 1], start=True, stop=True)

                # sigma_raw row: phi^T S_phi
                sc_bf = small.tile([P, ZU], BF16, tag="sc_bf")
                nc.scalar.copy(sc_bf[:], sphi[:])
                sig_ps = rpsum.tile([1, ZU], F32, tag="rows")
                nc.tensor.matmul(sig_ps[:], phiT_bf[:, s:s + 1], sc_bf[:],
                                 start=True, stop=True)
                nc.scalar.copy(sig_collect[:, s * ZU:(s + 1) * ZU], sig_ps[:])
                # mu row: ones^T (Q^T * S_phi)
                mt_bf = small.tile([P, ZU], BF16, tag="mt_bf")
                nc.vector.tensor_tensor(mt_bf[:], sphi[:], qt[:], op=ALU.mult)
                mu_ps = rpsum.tile([1, ZU], F32, tag="rows")
                nc.tensor.matmul(mu_ps[:], ones_bf[:], mt_bf[:],
                                 start=True, stop=True)
                nc.scalar.copy(mu_collect[:, s * ZU:(s + 1) * ZU], mu_ps[:])

            # cov = exp(logSigEps[u]) * (1 + sigma_raw); all on partition 0
            cov_row = const.tile([1, BS * ZU], F32)
            nc.vector.tensor_scalar_add(cov_row[:], sig_collect[:], 1.0)
            sigfac_b = sigf_row[:, None, None, :].broadcast_to([1, BS, Z, U])
            nc.vector.tensor_tensor(
                cov_row[:].rearrange("one (s z u) -> one s z u", s=BS, z=Z),
                cov_row[:].rearrange("one (s z u) -> one s z u", s=BS, z=Z),
                sigfac_b, op=ALU.mult)

            nc.sync.dma_start(out=mu_d[:].rearrange("s z u one -> (s z u one)")[None, :],
                              in_=mu_collect[:])
            nc.sync.dma_start(out=cov_d[:].rearrange("s z u -> (s z u)")[None, :],
                              in_=cov_row[:])

    nc.compile()
    return nc


_NC = None


def _get_nc():
    global _NC
    if _NC is None:
        _NC = build_nc()
    return _NC


def _in_maps(inputs):
    f = {k: np.ascontiguousarray(np.asarray(v, dtype=np.float32)) for k, v in inputs.items()}
    maps = []
    for c in range(NCORES):
        sl = slice(c * BS, (c + 1) * BS)
        maps.append({
            "x": f["x"][sl],
            "Linv": f["Linv"][sl],
            "Q": f["Q"][sl],
            "W1": f["W1"], "b1": f["b1"],
            "W2": f["W2"], "b2": f["b2"],
            "W3": f["W3"], "b3": f["b3"],
            "W4": f["W4"], "b4": f["b4"],
            "logSigEps": f["logSigEps"],
        })
    return maps


def kernel(**inputs):
    from concourse.bass_utils import run_bass_kernel_spmd

    nc = _get_nc()
    maps = _in_maps(inputs)
    res = run_bass_kernel_spmd(nc, maps, core_ids=list(range(NCORES)))
    mu = np.concatenate([np.asarray(res.results[i]["mu_out"]) for i in range(NCORES)], axis=0)
    cov = np.concatenate([np.asarray(res.results[i]["cov_out"]) for i in range(NCORES)], axis=0)
    return mu.astype(np.float32), cov.astype(np.float32)


# revision 6
# speedup vs baseline: 1.2896x; 1.2896x over previous
"""Trainium2 Bass kernel for the AlpacaMH head.

Math (per sample b, per (z,u) pair, A = Linv[b,z,u], 128x128):
    phi = MLP_encoder(x[b])                       # (P,)
    t_zu = A phi ; u_zu = A q_zu                  # one N=2 PE matvec
    sigma_raw[zu] = phi . t_zu   (= phi^T A phi)
    mu[zu]        = phi . u_zu   (= phi^T A q)
    cov[zu] = exp(logSigEps[u]) * (1 + sigma_raw[zu])

Strategy: pure data-parallel over batch across 8 NeuronCores (8 samples
per core).  The 256 MiB Linv tensor is streamed once from HBM in its
NATURAL CONTIGUOUS layout (16 KB per partition -> line-rate DMA
descriptors; a [i,j]-partitioned load would force 512 B descriptors and
~60% DMA efficiency), cast f32->bf16 inside the DMA (gpsimd SWDGE), and
repartitioned on-chip by PE transposes whose PSUM output is evacuated to
SBUF.  A strided access pattern over the transposed blocks reconstructs
each A_zu^T as a matmul stationary operand; rhs=[phi|q_zu] gives
[A phi | A q] columns in PSUM, and one final matmul with lhsT=phi
reduces a sample's S2[128,128] to the interleaved (sigma_raw, mu) row.
DMA of Linv is the roofline (~33.5 MB/core @ ~358 GB/s => ~94 us).
"""

import numpy as np

import concourse.bass as bass  # noqa: F401  (registers engine classes)
import concourse.mybir as mybir
import concourse.tile as tile
from concourse import bacc
from concourse.masks import make_identity

F32 = mybir.dt.float32
BF16 = mybir.dt.bfloat16
AF = mybir.ActivationFunctionType
ALU = mybir.AluOpType

# Problem dims (hardcoded per spec)
B, Z, U, P, X, H = 64, 8, 8, 128, 64, 512
NCORES = 8
BS = B // NCORES          # samples per core
ZU = Z * U                # 64 (z,u) pairs per sample
CZU = 32                  # zu pairs per flat chunk (2 MB f32)
NCHUNK = ZU // CZU        # chunks per sample


def build_nc():
    nc = bacc.Bacc(None, target_bir_lowering=False, debug=False)
    with tile.TileContext(nc) as tc:
        with (
            tc.tile_pool(name="dram", bufs=1, space="DRAM") as dram,
            tc.tile_pool(name="const", bufs=1) as const,
            tc.tile_pool(name="wts", bufs=1) as wts,
            tc.tile_pool(name="flat", bufs=3) as flatp,
            tc.tile_pool(name="obf", bufs=3) as obfp,
            tc.tile_pool(name="small", bufs=2) as small,
            tc.tile_pool(name="bigps", bufs=3, space="PSUM") as bigps,
            tc.tile_pool(name="s2ps", bufs=2, space="PSUM") as s2ps,
            tc.tile_pool(name="qps", bufs=1, space="PSUM") as qps,
            tc.tile_pool(name="rps", bufs=2, space="PSUM") as rps,
        ):
            # ---- DRAM parameters (names must match in_maps keys) ----
            x_d = dram.tile([BS, X], F32, kind="ExternalInput", name="x", uniquify=False)
            linv_d = dram.tile([BS, Z, U, P, P], F32, kind="ExternalInput", name="Linv", uniquify=False)
            q_d = dram.tile([BS, Z, U, 1, P], F32, kind="ExternalInput", name="Q", uniquify=False)
            w1_d = dram.tile([X, H], F32, kind="ExternalInput", name="W1", uniquify=False)
            b1_d = dram.tile([H], F32, kind="ExternalInput", name="b1", uniquify=False)
            w2_d = dram.tile([H, H], F32, kind="ExternalInput", name="W2", uniquify=False)
            b2_d = dram.tile([H], F32, kind="ExternalInput", name="b2", uniquify=False)
            w3_d = dram.tile([H, H], F32, kind="ExternalInput", name="W3", uniquify=False)
            b3_d = dram.tile([H], F32, kind="ExternalInput", name="b3", uniquify=False)
            w4_d = dram.tile([H, P], F32, kind="ExternalInput", name="W4", uniquify=False)
            b4_d = dram.tile([P], F32, kind="ExternalInput", name="b4", uniquify=False)
            ls_d = dram.tile([U], F32, kind="ExternalInput", name="logSigEps", uniquify=False)
            mu_d = dram.tile([BS, Z, U, 1], F32, kind="ExternalOutput", name="mu_out", uniquify=False)
            cov_d = dram.tile([BS, Z, U], F32, kind="ExternalOutput", name="cov_out", uniquify=False)

            # ---- constants / weights to SBUF ----
            ident = const.tile([128, 128], F32)
            make_identity(nc, ident[:])
            ident_bf = const.tile([128, 128], BF16)
            nc.vector.tensor_copy(ident_bf[:], ident[:])

            w1_sb = wts.tile([X, H], F32)
            nc.sync.dma_start(out=w1_sb[:], in_=w1_d[:])
            w2_sb = wts.tile([128, 4, H], F32)
            nc.sync.dma_start(out=w2_sb[:], in_=w2_d[:].rearrange("(kk p) h -> p kk h", p=128))
            w3_sb = wts.tile([128, 4, H], F32)
            nc.sync.dma_start(out=w3_sb[:], in_=w3_d[:].rearrange("(kk p) h -> p kk h", p=128))
            w4_sb = wts.tile([128, 4, P], F32)
            nc.sync.dma_start(out=w4_sb[:], in_=w4_d[:].rearrange("(kk p) h -> p kk h", p=128))
            b1_sb = wts.tile([128, 4], F32)
            nc.sync.dma_start(out=b1_sb[:], in_=b1_d[:].rearrange("(c p) -> p c", p=128))
            b2_sb = wts.tile([128, 4], F32)
            nc.sync.dma_start(out=b2_sb[:], in_=b2_d[:].rearrange("(c p) -> p c", p=128))
            b3_sb = wts.tile([128, 4], F32)
            nc.sync.dma_start(out=b3_sb[:], in_=b3_d[:].rearrange("(c p) -> p c", p=128))
            b4_sb = wts.tile([P, 1], F32)
            nc.sync.dma_start(out=b4_sb[:], in_=b4_d[:, None])

            ses = wts.tile([1, U], F32)
            nc.sync.dma_start(out=ses[:], in_=ls_d[None, :])
            sigf_row = const.tile([1, U], F32)
            nc.scalar.activation(sigf_row[:], ses[:], AF.Exp)

            # ---- encoder (transposed activations: [feature part, batch free])
            x_sb = small.tile([BS, X], F32, tag="x_sb")
            nc.sync.dma_start(out=x_sb[:], in_=x_d[:])
            xt_ps = bigps.tile([X, BS], F32, tag="t")
            nc.tensor.transpose(xt_ps[:], x_sb[:], ident[:BS, :BS])
            xt = small.tile([X, BS], F32, tag="xt")
            nc.scalar.copy(xt[:], xt_ps[:])

            def elu_into(pre_ps, bias_col, out_ap):
                # out = elu(pre + bias) = max(v, exp(min(v,0)) - 1)
                v = small.tile([128, BS], F32, tag="elu_v")
                nc.vector.tensor_scalar_add(v[:], pre_ps[:], bias_col)
                m = small.tile([128, BS], F32, tag="elu_m")
                nc.vector.tensor_scalar_min(m[:], v[:], 0.0)
                e = small.tile([128, BS], F32, tag="elu_e")
                nc.scalar.activation(e[:], m[:], AF.Exp)
                nc.vector.tensor_scalar_add(e[:], e[:], -1.0)
                nc.vector.tensor_tensor(out_ap, v[:], e[:], op=ALU.max)

            h1 = small.tile([128, 4, BS], F32, tag="h1")
            for m in range(4):
                ps = bigps.tile([128, BS], F32, tag="t")
                nc.tensor.matmul(ps[:], w1_sb[:, m * 128:(m + 1) * 128], xt[:],
                                 start=True, stop=True)
                elu_into(ps, b1_sb[:, m:m + 1], h1[:, m, :])
            h2 = small.tile([128, 4, BS], F32, tag="h2")
            for m in range(4):
                ps = bigps.tile([128, BS], F32, tag="t")
                for kk in range(4):
                    nc.tensor.matmul(ps[:], w2_sb[:, kk, m * 128:(m + 1) * 128],
                                     h1[:, kk, :], start=(kk == 0), stop=(kk == 3))
                elu_into(ps, b2_sb[:, m:m + 1], h2[:, m, :])
            h3 = small.tile([128, 4, BS], F32, tag="h3")
            for m in range(4):
                ps = bigps.tile([128, BS], F32, tag="t")
                for kk in range(4):
                    nc.tensor.matmul(ps[:], w3_sb[:, kk, m * 128:(m + 1) * 128],
                                     h2[:, kk, :], start=(kk == 0), stop=(kk == 3))
                elu_into(ps, b3_sb[:, m:m + 1], h3[:, m, :])

            phi_ps = bigps.tile([P, BS], F32, tag="t")
            for kk in range(4):
                nc.tensor.matmul(phi_ps[:], w4_sb[:, kk, :], h3[:, kk, :],
                                 start=(kk == 0), stop=(kk == 3))
            phiT = small.tile([P, BS], F32, tag="phiT")
            nc.vector.tensor_scalar_add(phiT[:], phi_ps[:], b4_sb[:])
            phiT_bf = const.tile([P, BS], BF16)
            nc.vector.tensor_copy(phiT_bf[:], phiT[:])

            # ---- main stream ----
            # flat view of Linv: [s, p(128), 4096] with 16 KB contiguous rows
            lvf = linv_d[:].rearrange("s z u i j -> s (z u i j)")
            qv = q_d[:].rearrange("s z u one j -> s (z u) (one j)")   # [BS, ZU, P]
            collect = const.tile([1, BS * 2 * ZU], F32)   # (s, zu, [sig|mu])

            for s in range(BS):
                # Q[s]^T: [zu, j] -> [j, zu] via PE transpose, then bf16
                qs = small.tile([ZU, P], F32, tag="qs")
                nc.sync.dma_start(out=qs[:], in_=qv[s])
                qt_ps = qps.tile([P, ZU], F32, tag="qt")
                nc.tensor.transpose(qt_ps[:], qs[:], ident[:ZU, :ZU])
                qt = small.tile([P, ZU], BF16, tag="qt_sb")
                nc.scalar.copy(qt[:], qt_ps[:])
                # rhs pairs [phi | q_zu] per zu
                rh = small.tile([P, ZU, 2], BF16, tag="rh")
                nc.vector.tensor_copy(rh[:, :, 0], phiT_bf[:, s:s + 1].broadcast_to([P, ZU]))
                nc.vector.tensor_copy(rh[:, :, 1], qt[:])

                s2 = s2ps.tile([P, 2 * ZU], F32, tag="s2")
                for h in range(NCHUNK):
                    tf = flatp.tile([128, 4096], BF16, tag="tf")
                    off = (h * CZU) * (P * P)
                    nc.gpsimd.dma_start(
                        out=tf[:],
                        in_=lvf[s, off:off + CZU * P * P].rearrange("(p f) -> p f", p=128))
                    # o2[j, p, c]: offset p*32+c => A_zl^T is the contiguous
                    # [128, 128] slice at free offset 128*zl
                    o2 = obfp.tile([128, 128, CZU], BF16, tag="obf")
                    for c4 in range(CZU // 4):
                        tp = bigps.tile([128, 512], BF16, tag="t")
                        for cc in range(4):
                            c = c4 * 4 + cc
                            nc.tensor.transpose(tp[:, cc * 128:(cc + 1) * 128],
                                                tf[:, c * 128:(c + 1) * 128], ident_bf[:])
                        dst = o2[:, :, c4 * 4:(c4 + 1) * 4]          # [128, 128, 4]
                        src = tp[:].rearrange("j (cc p) -> j p cc", cc=4)
                        if c4 % 2 == 0:
                            nc.vector.tensor_copy(dst, src)
                        else:
                            nc.scalar.copy(dst, src)
                    o2flat = o2[:].rearrange("j p c -> j (p c)")
                    for zl in range(CZU):
                        zu = h * CZU + zl
                        # lhsT = A_zl^T: [j part, i free], contiguous slice
                        nc.tensor.matmul(s2[:, 2 * zu:2 * zu + 2],
                                         o2flat[:, 128 * zl:128 * (zl + 1)],
                                         rh[:, zu, :], start=True, stop=True)

                sc = small.tile([P, 2 * ZU], BF16, tag="sc")
                nc.scalar.copy(sc[:], s2[:])
                row_ps = rps.tile([1, 2 * ZU], F32, tag="rows")
                nc.tensor.matmul(row_ps[:], phiT_bf[:, s:s + 1], sc[:],
                                 start=True, stop=True)
                nc.scalar.copy(collect[:, s * 2 * ZU:(s + 1) * 2 * ZU], row_ps[:])

            # ---- finalize: split sigma/mu, scale cov, store ----
            cview = collect[:].rearrange("one (s zu w) -> one s zu w", s=BS, zu=ZU)
            mu_row = const.tile([1, BS * ZU], F32)
            nc.vector.tensor_copy(mu_row[:].rearrange("one (s zu) -> one s zu", s=BS),
                                  cview[:, :, :, 1])
            cov_row = const.tile([1, BS * ZU], F32)
            nc.vector.tensor_scalar_add(
                cov_row[:].rearrange("one (s zu) -> one s zu", s=BS),
                cview[:, :, :, 0], 1.0)
            sigfac_b = sigf_row[:, None, None, :].broadcast_to([1, BS, Z, U])
            nc.vector.tensor_tensor(
                cov_row[:].rearrange("one (s z u) -> one s z u", s=BS, z=Z),
                cov_row[:].rearrange("one (s z u) -> one s z u", s=BS, z=Z),
                sigfac_b, op=ALU.mult)

            nc.sync.dma_start(out=mu_d[:].rearrange("s z u one -> (s z u one)")[None, :],
                              in_=mu_row[:])
            nc.sync.dma_start(out=cov_d[:].rearrange("s z u -> (s z u)")[None, :],
                              in_=cov_row[:])

    nc.compile()
    return nc


_NC = None


def _get_nc():
    global _NC
    if _NC is None:
        _NC = build_nc()
    return _NC


def _in_maps(inputs):
    f = {k: np.ascontiguousarray(np.asarray(v, dtype=np.float32)) for k, v in inputs.items()}
    maps = []
    for c in range(NCORES):
        sl = slice(c * BS, (c + 1) * BS)
        maps.append({
            "x": f["x"][sl],
            "Linv": f["Linv"][sl],
            "Q": f["Q"][sl],
            "W1": f["W1"], "b1": f["b1"],
            "W2": f["W2"], "b2": f["b2"],
            "W3": f["W3"], "b3": f["b3"],
            "W4": f["W4"], "b4": f["b4"],
            "logSigEps": f["logSigEps"],
        })
    return maps


def kernel(**inputs):
    from concourse.bass_utils import run_bass_kernel_spmd

    nc = _get_nc()
    maps = _in_maps(inputs)
    res = run_bass_kernel_spmd(nc, maps, core_ids=list(range(NCORES)))
    mu = np.concatenate([np.asarray(res.results[i]["mu_out"]) for i in range(NCORES)], axis=0)
    cov = np.concatenate([np.asarray(res.results[i]["cov_out"]) for i in range(NCORES)], axis=0)
    return mu.astype(np.float32), cov.astype(np.float32)
